# revision 2
# baseline (speedup 1.0000x reference)
"""Trainium2 Bass kernel for nn_MultiHeadAttention_62319975465542.

Tensor-parallel over heads (Megatron-style): 32 heads sharded 4-per-core
across 8 NeuronCores; host sums the 8 partial out-projections + bias.

v5 over v3:
- Q/K projections run in fp8e4m3 with MatmulPerfMode.DoubleRow (2
  k-tiles per pass, 0.5 cycles/row): 4x fewer PE cycles there. Scores /
  A*V / V / out-projection stay bf16: measured on silicon, an
  idle-punctuated PE (which an fp8-fast score phase causes, since exp on
  ScalarE then paces the pipeline) drops the PE DVFS ladder from 2.4GHz
  to ~2.0GHz and taxes ALL matmuls 20%, so bf16 scores that keep the PE
  saturated are net faster.
- Score scale 1/sqrt(D) is folded into the Q store (mult+add
  tensor_scalar); weights host-scaled by 8 for fp8 normal range.
- exp is emitted earlier (scores before V-proj in the lead-in loop).
- Y partials output in bf16 (halves output DMA).
"""

import numpy as np
import ml_dtypes

import concourse.bass as bass
import concourse.tile as tile
import concourse.mybir as mybir
from concourse import bacc

F32 = mybir.dt.float32
BF16 = mybir.dt.bfloat16
FP8 = mybir.dt.float8e4
DR = mybir.MatmulPerfMode.DoubleRow
Identity = mybir.ActivationFunctionType.Identity
Exp = mybir.ActivationFunctionType.Exp
MULT = mybir.AluOpType.mult
ADD = mybir.AluOpType.add

L = 2048          # sequence length
D = 2048          # d_model
NH = 32           # total heads
DH = 64           # head dim
NCORES = 8
HPC = NH // NCORES   # heads per core = 4
JC = HPC * DH        # per-core projected width = 256
LB = 512             # l-block width
NLB = L // LB        # 4
KP = 8               # fp8 contraction k-pair chunks (2048 = 8 * 2 * 128)
KO = D // 128        # 16 bf16 contraction chunks (V proj)
MC = L // 128        # 16 key chunks
W8S = 8.0            # fp8 weights host-scaled by 8 for normal range
QS = float(D) ** -0.5 / W8S   # q store scale: 1/sqrt(D) / 8
KS = 1.0 / W8S


def build_program():
    nc = bacc.Bacc("TRN2", target_bir_lowering=False, debug=False)

    xt8_d = nc.dram_tensor("XT8", (128, KP, 2, L), FP8, kind="ExternalInput")
    xt_d = nc.dram_tensor("XT", (128, KO, NLB, LB), BF16, kind="ExternalInput")
    wq8_d = nc.dram_tensor("WQ8", (128, KP, 2, JC), FP8, kind="ExternalInput")
    wk8_d = nc.dram_tensor("WK8", (128, KP, 2, JC), FP8, kind="ExternalInput")
    wv_d = nc.dram_tensor("WV", (128, KO, JC), BF16, kind="ExternalInput")
    wo_d = nc.dram_tensor("WO", (128, 2, D), BF16, kind="ExternalInput")
    bq_d = nc.dram_tensor("BQ", (128, 2), F32, kind="ExternalInput")
    bk_d = nc.dram_tensor("BK", (128, 2), F32, kind="ExternalInput")
    bv_d = nc.dram_tensor("BV", (1, JC), BF16, kind="ExternalInput")
    ones_d = nc.dram_tensor("ONES", (1, 128), BF16, kind="ExternalInput")
    y_d = nc.dram_tensor("Y", (L, D), BF16, kind="ExternalOutput")

    with tile.TileContext(nc) as tc, nc.allow_low_precision(
            reason="fp8 QK path + bf16 V path are within tolerance"):
        with (
            tc.tile_pool(name="const", bufs=1) as cp,
            tc.tile_pool(name="xtp", bufs=2) as xtpool,
            tc.tile_pool(name="epool", bufs=20) as epool,
            tc.tile_pool(name="norm", bufs=1) as normp,
            tc.tile_pool(name="ysb", bufs=3) as ypool,
            tc.tile_pool(name="scps", bufs=2, space="PSUM") as scps,
            tc.tile_pool(name="accps", bufs=4, space="PSUM") as accps,
        ):
            wq8_sb = cp.tile((128, KP, 2, JC), FP8)
            wk8_sb = cp.tile((128, KP, 2, JC), FP8)
            wv_sb = cp.tile((128, KO, JC), BF16)
            wo_sb = cp.tile((128, 2, D), BF16)
            bq_sb = cp.tile((128, 2), F32)
            bk_sb = cp.tile((128, 2), F32)
            bv_sb = cp.tile((1, JC), BF16)
            ones_sb = cp.tile((1, 128), BF16)
            xt8_sb = cp.tile((128, KP, 2, L), FP8)
            xtb = {}    # rotating per-l-block bf16 X tiles (V-proj only)
            # qt/kt: pair p holds heads {2p, 2p+1} as partition halves
            qt_sb = [cp.tile((128, L), BF16, name=f"qt{p}") for p in range(2)]
            kt_sb = [cp.tile((128, L), BF16, name=f"kt{p}") for p in range(2)]
            v_sb = cp.tile((128, MC, HPC * 65), BF16)
            ot_sb = [cp.tile((128, L), BF16, name=f"ot{p}") for p in range(2)]

            nc.vector.memset(v_sb[:], 1.0)

            # DMA in urgency order: the Q(lb0,jc0)/K(mb0,jc0) projections and
            # first score chunks only need WQ8+BQ8+xt8(lb0)+WK8+BK8 (~2.5MB).
            def load_xt8(lb):
                for k in range(KP):
                    nc.sync.dma_start(
                        xt8_sb[:, k, :, lb * LB:(lb + 1) * LB],
                        xt8_d[:, k, :, lb * LB:(lb + 1) * LB],
                    )

            def load_xt(lb):
                xtb[lb] = xtpool.tile((128, KO, LB), BF16, name="xt_lb")
                for ko in range(KO):
                    nc.sync.dma_start(
                        xtb[lb][:, ko, :],
                        xt_d[:, ko, lb, :],
                    )

            nc.sync.dma_start(wq8_sb[:, :, :, 0:128], wq8_d[:, :, :, 0:128])
            nc.sync.dma_start(bq_sb[:], bq_d[:])
            load_xt8(0)
            nc.sync.dma_start(wk8_sb[:, :, :, 0:128], wk8_d[:, :, :, 0:128])
            nc.sync.dma_start(bk_sb[:], bk_d[:])
            nc.sync.dma_start(wq8_sb[:, :, :, 128:256], wq8_d[:, :, :, 128:256])
            nc.sync.dma_start(wk8_sb[:, :, :, 128:256], wk8_d[:, :, :, 128:256])
            nc.sync.dma_start(wv_sb[:], wv_d[:])
            nc.sync.dma_start(bv_sb[:], bv_d[:])
            nc.sync.dma_start(ones_sb[:], ones_d[:])
            load_xt(0)
            for lb in range(1, NLB):
                load_xt8(lb)
            for lb in range(1, NLB):
                load_xt(lb)
            nc.sync.dma_start(wo_sb[:], wo_d[:])

            def proj_qk8(w_sb, b_sb, dst, lb, jc, scale):
                """fp8 DoubleRow projection of one (jc, lb) block; PSUM ->
                bf16 store with x*scale + bias (bias pre-scaled on host)."""
                ps = accps.tile((128, LB), F32, name="acc_ps")
                for k in range(KP):
                    nc.tensor.matmul(
                        ps[:],
                        w_sb[:, k, :, jc * 128:(jc + 1) * 128],
                        xt8_sb[:, k, :, lb * LB:(lb + 1) * LB],
                        start=(k == 0), stop=(k == KP - 1),
                        perf_mode=DR,
                    )
                nc.vector.tensor_scalar(
                    dst[jc][:, lb * LB:(lb + 1) * LB], ps[:],
                    scale, b_sb[:, jc:jc + 1], MULT, ADD,
                )

            def proj_v(lb):
                """V in (l, j) layout; bias via K=1 ones-row matmul."""
                for lt in range(4):
                    vp = accps.tile((128, LB), F32, name="acc_ps")
                    for ko in range(KO):
                        nc.tensor.matmul(
                            vp[:, 0:JC],
                            xtb[lb][:, ko, lt * 128:(lt + 1) * 128],
                            wv_sb[:, ko, :],
                            start=(ko == 0), stop=False,
                        )
                    nc.tensor.matmul(
                        vp[:, 0:JC], ones_sb[0:1, :], bv_sb[0:1, :],
                        start=False, stop=True,
                    )
                    nc.vector.tensor_copy(
                        v_sb[:, lb * 4 + lt, :].rearrange(
                            "p (h e) -> p h e", h=HPC)[:, :, 0:DH],
                        vp[:, 0:JC].rearrange("p (h d) -> p h d", h=HPC))

            def attn_pair_start():
                ava = accps.tile((128, LB), F32, name="acc_ps")
                avb = accps.tile((128, LB), F32, name="acc_ps")
                return ava, avb

            def score_chunk(lb, p, m):
                """bf16 scores + exp for one (m, l-block) chunk; returns
                the bf16 exp tile (consumed by a later A·V)."""
                lsl = slice(lb * LB, (lb + 1) * LB)
                msl = slice(m * 128, (m + 1) * 128)
                sc = scps.tile((128, 2 * LB), F32, name="sc_ps")
                nc.tensor.matmul(
                    sc[:, 0:LB],
                    kt_sb[p][0:64, msl], qt_sb[p][0:64, lsl],
                )
                nc.tensor.matmul(
                    sc[:, LB:2 * LB],
                    kt_sb[p][64:128, msl], qt_sb[p][64:128, lsl],
                )
                e = epool.tile((128, 2 * LB), BF16, name="e_sb")
                nc.scalar.activation(e[:], sc[:], Exp)
                return e

            def av_chunk(p, m, ava, avb, e):
                # A·V per head, [V_h | 1] lhsT: row 64 = denominator
                nc.tensor.matmul(
                    ava[0:65, :],
                    v_sb[:, m, (2 * p) * 65:(2 * p) * 65 + 65],
                    e[:, 0:LB],
                    start=(m == 0), stop=(m == MC - 1),
                )
                nc.tensor.matmul(
                    avb[0:65, :],
                    v_sb[:, m, (2 * p + 1) * 65:(2 * p + 1) * 65 + 65],
                    e[:, LB:2 * LB],
                    start=(m == 0), stop=(m == MC - 1),
                )

            def attn_pair_finish(lb, p, ava, avb):
                # copy raw A·V (+denominator row) to SBUF, freeing the
                # PSUM accumulators immediately; normalize from SBUF
                lsl = slice(lb * LB, (lb + 1) * LB)
                sva = normp.tile((65, LB), F32, name="sva_sb")
                svb = normp.tile((65, LB), F32, name="svb_sb")
                nc.any.tensor_copy(sva[:], ava[0:65, :])
                nc.any.tensor_copy(svb[:], avb[0:65, :])
                ra = normp.tile((1, LB), F32, name="ra_sb")
                rb = normp.tile((1, LB), F32, name="rb_sb")
                nc.vector.reciprocal(ra[:], sva[64:65, :])
                nc.vector.reciprocal(rb[:], svb[64:65, :])
                rba = normp.tile((64, LB), F32, name="rba_sb")
                rbb = normp.tile((64, LB), F32, name="rbb_sb")
                nc.gpsimd.partition_broadcast(rba[:], ra[:])
                nc.gpsimd.partition_broadcast(rbb[:], rb[:])
                nc.vector.tensor_tensor(
                    ot_sb[p][0:64, lsl], sva[0:64, :], rba[:], MULT)
                nc.vector.tensor_tensor(
                    ot_sb[p][64:128, lsl], svb[0:64, :], rbb[:], MULT)

            def outproj_tile(lb, lt, ns):
                row0 = lb * LB + lt * 128
                yp = accps.tile((128, 512), F32, name="acc_ps")
                for jc in range(2):
                    nc.tensor.matmul(
                        yp[:],
                        ot_sb[jc][:, row0:row0 + 128],
                        wo_sb[:, jc, ns * 512:(ns + 1) * 512],
                        start=(jc == 0), stop=(jc == 1),
                    )
                ty = ypool.tile((128, 512), BF16, name="y_sb")
                nc.any.tensor_copy(ty[:], yp[:])
                nc.sync.dma_start(
                    y_d[row0:row0 + 128, ns * 512:(ns + 1) * 512], ty[:])

            # ── Emission schedule ────────────────────────────────────────
            # Streams S_i = (lb, p). Scores of stream i overlap A·V of
            # stream i-1 on PE; exp (ScalarE) is the pacing engine, so
            # scores are emitted as early as the projections allow.
            ava0, avb0 = attn_pair_start()
            e1 = []
            proj_qk8(wq8_sb, bq_sb, qt_sb, 0, 0, QS)
            for mb in range(NLB):
                proj_qk8(wk8_sb, bk_sb, kt_sb, mb, 0, KS)
                s0 = [score_chunk(0, 0, m) for m in range(4 * mb, 4 * mb + 4)]
                if mb == 0:
                    proj_qk8(wq8_sb, bq_sb, qt_sb, 0, 1, QS)
                proj_qk8(wk8_sb, bk_sb, kt_sb, mb, 1, KS)
                e1 += [score_chunk(0, 1, m) for m in range(4 * mb, 4 * mb + 4)]
                proj_v(mb)
                for i, m in enumerate(range(4 * mb, 4 * mb + 4)):
                    av_chunk(0, m, ava0, avb0, s0[i])
            attn_pair_finish(0, 0, ava0, avb0)

            # Q(lb1) must be resident before stream (1,0)'s scores
            proj_qk8(wq8_sb, bq_sb, qt_sb, 1, 0, QS)
            proj_qk8(wq8_sb, bq_sb, qt_sb, 1, 1, QS)

            filler = [
                (lambda lb=lb, jc=jc: proj_qk8(wq8_sb, bq_sb, qt_sb, lb, jc, QS))
                for lb in range(2, NLB) for jc in range(2)
            ]
            prev = (0, 1, e1)    # stream whose A·V phase is pending
            pending = []         # fillers gated on a not-yet-finished chain
            streams = [(lb, p) for lb in range(1, NLB) for p in range(2)]
            for i, (lb, p) in enumerate(streams):
                last = i == len(streams) - 1
                e_list = []
                pava, pavb = attn_pair_start()
                for m in range(MC):
                    if m == 8 and pending:
                        # one half-iteration after the finish that queued
                        # them: the normalize chain has drained by now, so
                        # these no longer stall the in-order PE queue
                        filler.extend(pending)
                        pending = []
                    e_list.append(score_chunk(lb, p, m))
                    av_chunk(prev[1], m, pava, pavb, prev[2][m])
                    if filler and (last or m % 2 == 0):
                        filler.pop(0)()
                attn_pair_finish(prev[0], prev[1], pava, pavb)
                if prev[1] == 1:
                    flb = prev[0]
                    pending += [
                        (lambda flb=flb, lt=lt, ns=ns:
                         outproj_tile(flb, lt, ns))
                        for lt in range(4) for ns in range(4)
                    ]
                prev = (lb, p, e_list)
            # tail: A·V of the last stream (PE-dense, no pops), normalize,
            # then leftover fillers (they cover the final chain), then the
            # last out-projection
            pava, pavb = attn_pair_start()
            filler.extend(pending)
            for m in range(MC):
                av_chunk(prev[1], m, pava, pavb, prev[2][m])
            attn_pair_finish(prev[0], prev[1], pava, pavb)
            for f in filler:
                f()
            for lt in range(4):
                for ns in range(4):
                    outproj_tile(NLB - 1, lt, ns)

    nc.compile()
    return nc


def make_core_inputs(X, Wq_w, Wq_b, Wk_w, Wk_b, Wv_w, Wv_b, Wo_w):
    """Host-side sharding: per-core input dicts (shared X + per-core weights)."""
    X = np.asarray(X, np.float32)
    bf = ml_dtypes.bfloat16
    f8 = ml_dtypes.float8_e4m3fn
    xt = np.ascontiguousarray(X.T)
    xt_bf = np.ascontiguousarray(
        xt.reshape(KO, 128, L).transpose(1, 0, 2)).astype(bf).reshape(
            128, KO, NLB, LB)
    xt_f8 = np.ascontiguousarray(
        xt.reshape(KP, 2, 128, L).transpose(2, 0, 1, 3)).astype(f8)
    in_maps = []
    for c in range(NCORES):
        idx = np.array([d * NH + h for h in range(c * HPC, (c + 1) * HPC)
                        for d in range(DH)], np.int64)

        def kxj8(w):
            # (D_in, JC) -> (128, KP, 2, JC): [p,k,i,j] = 8*w.T[(2k+i)*128+p, j]
            wt = np.ascontiguousarray(
                (np.asarray(w, np.float32)[idx, :] * W8S).T)
            return np.ascontiguousarray(
                wt.reshape(KP, 2, 128, JC).transpose(2, 0, 1, 3)).astype(f8)

        def kxj(w):
            wt = np.ascontiguousarray(np.asarray(w, np.float32)[idx, :].T)
            return np.ascontiguousarray(
                wt.reshape(KO, 128, JC).transpose(1, 0, 2)).astype(bf)

        wo = np.ascontiguousarray(np.asarray(Wo_w, np.float32)[:, idx].T)
        wo = np.ascontiguousarray(
            wo.reshape(2, 128, D).transpose(1, 0, 2)).astype(bf)

        def bcol(b, s):
            return np.ascontiguousarray(
                (np.asarray(b, np.float32)[idx] * s).reshape(2, 128).T)

        in_maps.append({
            "XT8": xt_f8, "XT": xt_bf,
            "WQ8": kxj8(Wq_w), "WK8": kxj8(Wk_w), "WV": kxj(Wv_w),
            "WO": wo,
            "BQ": bcol(Wq_b, float(D) ** -0.5), "BK": bcol(Wk_b, 1.0),
            "BV": np.asarray(Wv_b, np.float32)[idx].reshape(1, JC).astype(bf),
            "ONES": np.ones((1, 128), bf),
        })
    return in_maps


_prog_cache = {}


def kernel(X, Wq_w, Wq_b, Wk_w, Wk_b, Wv_w, Wv_b, Wo_w, Wo_b, _trace=False):
    from concourse.bass_utils import run_bass_kernel_spmd

    if "nc" not in _prog_cache:
        _prog_cache["nc"] = build_program()
    nc = _prog_cache["nc"]
    in_maps = make_core_inputs(X, Wq_w, Wq_b, Wk_w, Wk_b, Wv_w, Wv_b, Wo_w)
    res = run_bass_kernel_spmd(nc, in_maps, core_ids=list(range(NCORES)),
                               trace=_trace)
    y = np.zeros((L, D), np.float64)
    for r in res.results:
        y += np.asarray(r["Y"]).astype(np.float64)
    y += np.asarray(Wo_b, np.float32).astype(np.float64)
    out = y.astype(np.float32)
    if _trace:
        kernel.last_results = res
    return out


# revision 3
# speedup vs baseline: 1.0429x; 1.0429x over previous
"""Trainium2 Bass kernel for nn_MultiHeadAttention_62319975465542 (v4).

Tensor-parallel over heads (Megatron-style): 32 heads sharded 4-per-core
across 8 NeuronCores; host sums the 8 partial out-projections + bias.

v5 over v3:
- Q/K projections run in fp8e4m3 with MatmulPerfMode.DoubleRow (2
  k-tiles per pass, 0.5 cycles/row): 4x fewer PE cycles there. Scores /
  A*V / V / out-projection stay bf16: measured on silicon, an
  idle-punctuated PE (which an fp8-fast score phase causes, since exp on
  ScalarE then paces the pipeline) drops the PE DVFS ladder from 2.4GHz
  to ~2.0GHz and taxes ALL matmuls 20%, so bf16 scores that keep the PE
  saturated are net faster.
- Score scale 1/sqrt(D) is folded into the Q store (mult+add
  tensor_scalar); weights host-scaled by 8 for fp8 normal range.
- exp is emitted earlier (scores before V-proj in the lead-in loop).
- Y partials output in bf16 (halves output DMA).
"""

import numpy as np
import ml_dtypes

import concourse.bass as bass
import concourse.tile as tile
import concourse.mybir as mybir
from concourse import bacc

F32 = mybir.dt.float32
BF16 = mybir.dt.bfloat16
FP8 = mybir.dt.float8e4
DR = mybir.MatmulPerfMode.DoubleRow
Identity = mybir.ActivationFunctionType.Identity
Exp = mybir.ActivationFunctionType.Exp
MULT = mybir.AluOpType.mult
ADD = mybir.AluOpType.add

L = 2048          # sequence length
D = 2048          # d_model
NH = 32           # total heads
DH = 64           # head dim
NCORES = 8
HPC = NH // NCORES   # heads per core = 4
JC = HPC * DH        # per-core projected width = 256
LB = 512             # l-block width
NLB = L // LB        # 4
KP = 8               # fp8 contraction k-pair chunks (2048 = 8 * 2 * 128)
KO = D // 128        # 16 bf16 contraction chunks (V proj)
MC = L // 128        # 16 key chunks
W8S = 8.0            # fp8 weights host-scaled by 8 for normal range
QS = float(D) ** -0.5 / W8S   # q store scale: 1/sqrt(D) / 8
KS = 1.0 / W8S


def build_program():
    nc = bacc.Bacc("TRN2", target_bir_lowering=False, debug=False)

    xt8_d = nc.dram_tensor("XT8", (128, KP, 2, L), FP8, kind="ExternalInput")
    xt_d = nc.dram_tensor("XT", (128, KO, NLB, LB), BF16, kind="ExternalInput")
    wq8_d = nc.dram_tensor("WQ8", (128, KP, 2, JC), FP8, kind="ExternalInput")
    wk8_d = nc.dram_tensor("WK8", (128, KP, 2, JC), FP8, kind="ExternalInput")
    wv_d = nc.dram_tensor("WV", (128, KO, JC), BF16, kind="ExternalInput")
    wo_d = nc.dram_tensor("WO", (128, 2, D), BF16, kind="ExternalInput")
    bq_d = nc.dram_tensor("BQ", (128, 2), F32, kind="ExternalInput")
    bk_d = nc.dram_tensor("BK", (128, 2), F32, kind="ExternalInput")
    bv_d = nc.dram_tensor("BV", (1, JC), BF16, kind="ExternalInput")
    ones_d = nc.dram_tensor("ONES", (1, 128), BF16, kind="ExternalInput")
    y_d = nc.dram_tensor("Y", (L, D), BF16, kind="ExternalOutput")

    with tile.TileContext(nc) as tc, nc.allow_low_precision(
            reason="fp8 QK path + bf16 V path are within tolerance"):
        with (
            tc.tile_pool(name="const", bufs=1) as cp,
            tc.tile_pool(name="xtp", bufs=2) as xtpool,
            tc.tile_pool(name="epool", bufs=20) as epool,
            tc.tile_pool(name="norm", bufs=1) as normp,
            tc.tile_pool(name="ysb", bufs=3) as ypool,
            tc.tile_pool(name="scps", bufs=2, space="PSUM") as scps,
            tc.tile_pool(name="accps", bufs=4, space="PSUM") as accps,
        ):
            wq8_sb = cp.tile((128, KP, 2, JC), FP8)
            wk8_sb = cp.tile((128, KP, 2, JC), FP8)
            wv_sb = cp.tile((128, KO, JC), BF16)
            wo_sb = cp.tile((128, 2, D), BF16)
            bq_sb = cp.tile((128, 2), F32)
            bk_sb = cp.tile((128, 2), F32)
            bv_sb = cp.tile((1, JC), BF16)
            ones_sb = cp.tile((1, 128), BF16)
            xt8_sb = cp.tile((128, KP, 2, L), FP8)
            xtb = {}    # rotating per-l-block bf16 X tiles (V-proj only)
            # qt/kt: pair p holds heads {2p, 2p+1} as partition halves
            qt_sb = [cp.tile((128, L), BF16, name=f"qt{p}") for p in range(2)]
            kt_sb = [cp.tile((128, L), BF16, name=f"kt{p}") for p in range(2)]
            v_sb = cp.tile((128, MC, HPC * 65), BF16)
            ot_sb = [cp.tile((128, L), BF16, name=f"ot{p}") for p in range(2)]

            nc.vector.memset(v_sb[:], 1.0)

            # DMA in urgency order: the Q(lb0,jc0)/K(mb0,jc0) projections and
            # first score chunks only need WQ8+BQ8+xt8(lb0)+WK8+BK8 (~2.5MB).
            def load_xt8(lb, eng):
                for k in range(0, KP, 2):
                    eng.dma_start(
                        xt8_sb[:, k:k + 2, :, lb * LB:(lb + 1) * LB],
                        xt8_d[:, k:k + 2, :, lb * LB:(lb + 1) * LB],
                    )

            def load_xt(lb, eng):
                xtb[lb] = xtpool.tile((128, KO, LB), BF16, name="xt_lb")
                for ko in range(0, KO, 4):
                    eng.dma_start(
                        xtb[lb][:, ko:ko + 4, :],
                        xt_d[:, ko:ko + 4, lb, :],
                    )

            # all loads on the sync (SP) hardware-DGE queue, in strict
            # priority order: SP issues ~565ns apart, queues round-robin,
            # so the first compute's bytes lead every queue.
            nc.sync.dma_start(wq8_sb[:, :, :, 0:128], wq8_d[:, :, :, 0:128])
            nc.sync.dma_start(bq_sb[:], bq_d[:])
            load_xt8(0, nc.sync)
            nc.sync.dma_start(wk8_sb[:, :, :, 0:128], wk8_d[:, :, :, 0:128])
            nc.sync.dma_start(bk_sb[:], bk_d[:])
            nc.sync.dma_start(wv_sb[:], wv_d[:])
            nc.sync.dma_start(bv_sb[:], bv_d[:])
            nc.sync.dma_start(ones_sb[:], ones_d[:])
            load_xt(0, nc.sync)
            nc.sync.dma_start(wq8_sb[:, :, :, 128:256],
                              wq8_d[:, :, :, 128:256])
            nc.sync.dma_start(wk8_sb[:, :, :, 128:256],
                              wk8_d[:, :, :, 128:256])
            load_xt8(1, nc.sync)
            load_xt(1, nc.sync)
            for lb in range(2, NLB):
                load_xt8(lb, nc.sync)
                load_xt(lb, nc.sync)
            nc.sync.dma_start(wo_sb[:], wo_d[:])

            def proj_qk8(w_sb, b_sb, dst, lb, jc, scale):
                """fp8 DoubleRow projection of one (jc, lb) block; PSUM ->
                bf16 store with x*scale + bias (bias pre-scaled on host)."""
                ps = accps.tile((128, LB), F32, name="acc_ps")
                for k in range(KP):
                    nc.tensor.matmul(
                        ps[:],
                        w_sb[:, k, :, jc * 128:(jc + 1) * 128],
                        xt8_sb[:, k, :, lb * LB:(lb + 1) * LB],
                        start=(k == 0), stop=(k == KP - 1),
                        perf_mode=DR,
                    )
                nc.vector.tensor_scalar(
                    dst[jc][:, lb * LB:(lb + 1) * LB], ps[:],
                    scale, b_sb[:, jc:jc + 1], MULT, ADD,
                )

            def proj_v(lb):
                """V in (l, j) layout; bias via K=1 ones-row matmul."""
                for lt in range(4):
                    vp = accps.tile((128, LB), F32, name="acc_ps")
                    for ko in range(KO):
                        nc.tensor.matmul(
                            vp[:, 0:JC],
                            xtb[lb][:, ko, lt * 128:(lt + 1) * 128],
                            wv_sb[:, ko, :],
                            start=(ko == 0), stop=False,
                        )
                    nc.tensor.matmul(
                        vp[:, 0:JC], ones_sb[0:1, :], bv_sb[0:1, :],
                        start=False, stop=True,
                    )
                    nc.vector.tensor_copy(
                        v_sb[:, lb * 4 + lt, :].rearrange(
                            "p (h e) -> p h e", h=HPC)[:, :, 0:DH],
                        vp[:, 0:JC].rearrange("p (h d) -> p h d", h=HPC))

            def attn_pair_start():
                ava = accps.tile((128, LB), F32, name="acc_ps")
                avb = accps.tile((128, LB), F32, name="acc_ps")
                return ava, avb

            def score_chunk(lb, p, m):
                """bf16 scores + exp for one (m, l-block) chunk; returns
                the bf16 exp tile (consumed by a later A·V)."""
                lsl = slice(lb * LB, (lb + 1) * LB)
                msl = slice(m * 128, (m + 1) * 128)
                sc = scps.tile((128, 2 * LB), F32, name="sc_ps")
                nc.tensor.matmul(
                    sc[:, 0:LB],
                    kt_sb[p][0:64, msl], qt_sb[p][0:64, lsl],
                )
                nc.tensor.matmul(
                    sc[:, LB:2 * LB],
                    kt_sb[p][64:128, msl], qt_sb[p][64:128, lsl],
                )
                e = epool.tile((128, 2 * LB), BF16, name="e_sb")
                nc.scalar.activation(e[:], sc[:], Exp)
                return e

            def av_chunk(p, m, ava, avb, e):
                # A·V per head, [V_h | 1] lhsT: row 64 = denominator
                nc.tensor.matmul(
                    ava[0:65, :],
                    v_sb[:, m, (2 * p) * 65:(2 * p) * 65 + 65],
                    e[:, 0:LB],
                    start=(m == 0), stop=(m == MC - 1),
                )
                nc.tensor.matmul(
                    avb[0:65, :],
                    v_sb[:, m, (2 * p + 1) * 65:(2 * p + 1) * 65 + 65],
                    e[:, LB:2 * LB],
                    start=(m == 0), stop=(m == MC - 1),
                )

            def attn_pair_finish(lb, p, ava, avb):
                # copy raw A·V (+denominator row) to SBUF, freeing the
                # PSUM accumulators immediately; normalize from SBUF
                lsl = slice(lb * LB, (lb + 1) * LB)
                sva = normp.tile((65, LB), F32, name="sva_sb")
                svb = normp.tile((65, LB), F32, name="svb_sb")
                nc.vector.tensor_copy(sva[:], ava[0:65, :])
                nc.vector.tensor_copy(svb[:], avb[0:65, :])
                # one reciprocal covers both denominators: rows staged at
                # partitions 0 (from sva) and 64 (from svb); DVE op cost is
                # free-size driven so (65,LB) costs the same as (1,LB).
                # partition_broadcast sources must sit at partition 0, so
                # rr[64] is re-staged into rb first.
                dn = normp.tile((65, LB), F32, name="dn_sb")
                nc.vector.tensor_copy(dn[0:1, :], sva[64:65, :])
                nc.vector.tensor_copy(dn[64:65, :], svb[64:65, :])
                rr = normp.tile((65, LB), F32, name="rr_sb")
                nc.vector.reciprocal(rr[:], dn[:])
                rb = normp.tile((1, LB), F32, name="rb_sb")
                nc.vector.tensor_copy(rb[:], rr[64:65, :])
                rba = normp.tile((64, LB), F32, name="rba_sb")
                rbb = normp.tile((64, LB), F32, name="rbb_sb")
                nc.gpsimd.partition_broadcast(rba[:], rr[0:1, :])
                nc.gpsimd.partition_broadcast(rbb[:], rb[:])
                nc.vector.tensor_tensor(
                    ot_sb[p][0:64, lsl], sva[0:64, :], rba[:], MULT)
                nc.vector.tensor_tensor(
                    ot_sb[p][64:128, lsl], svb[0:64, :], rbb[:], MULT)

            def outproj_tile(lb, lt, ns):
                row0 = lb * LB + lt * 128
                yp = accps.tile((128, 512), F32, name="acc_ps")
                for jc in range(2):
                    nc.tensor.matmul(
                        yp[:],
                        ot_sb[jc][:, row0:row0 + 128],
                        wo_sb[:, jc, ns * 512:(ns + 1) * 512],
                        start=(jc == 0), stop=(jc == 1),
                    )
                ty = ypool.tile((128, 512), BF16, name="y_sb")
                nc.vector.tensor_copy(ty[:], yp[:])
                nc.sync.dma_start(
                    y_d[row0:row0 + 128, ns * 512:(ns + 1) * 512], ty[:])

            # ── Emission schedule ────────────────────────────────────────
            # Streams S_i = (lb, p). Scores of stream i overlap A·V of
            # stream i-1 on PE; exp (ScalarE) is the pacing engine, so
            # scores are emitted as early as the projections allow.
            ava0, avb0 = attn_pair_start()
            e1 = []
            proj_qk8(wq8_sb, bq_sb, qt_sb, 0, 0, QS)
            for mb in range(NLB):
                proj_qk8(wk8_sb, bk_sb, kt_sb, mb, 0, KS)
                s0 = [score_chunk(0, 0, m) for m in range(4 * mb, 4 * mb + 4)]
                if mb == 0:
                    proj_qk8(wq8_sb, bq_sb, qt_sb, 0, 1, QS)
                proj_qk8(wk8_sb, bk_sb, kt_sb, mb, 1, KS)
                e1 += [score_chunk(0, 1, m) for m in range(4 * mb, 4 * mb + 4)]
                proj_v(mb)
                for i, m in enumerate(range(4 * mb, 4 * mb + 4)):
                    av_chunk(0, m, ava0, avb0, s0[i])
            attn_pair_finish(0, 0, ava0, avb0)

            # Q(lb1) must be resident before stream (1,0)'s scores
            proj_qk8(wq8_sb, bq_sb, qt_sb, 1, 0, QS)
            proj_qk8(wq8_sb, bq_sb, qt_sb, 1, 1, QS)

            filler = [
                (lambda lb=lb, jc=jc: proj_qk8(wq8_sb, bq_sb, qt_sb, lb, jc, QS))
                for lb in range(2, NLB) for jc in range(2)
            ]
            prev = (0, 1, e1)    # stream whose A·V phase is pending
            pending = []         # fillers gated on a not-yet-finished chain
            streams = [(lb, p) for lb in range(1, NLB) for p in range(2)]
            for i, (lb, p) in enumerate(streams):
                last = i == len(streams) - 1
                e_list = []
                pava, pavb = attn_pair_start()
                for m in range(MC):
                    if m == 12 and pending:
                        # one half-iteration after the finish that queued
                        # them: the normalize chain has drained by now, so
                        # these no longer stall the in-order PE queue
                        filler.extend(pending)
                        pending = []
                    e_list.append(score_chunk(lb, p, m))
                    av_chunk(prev[1], m, pava, pavb, prev[2][m])
                    if filler and (last or m % 2 == 0):
                        filler.pop(0)()
                attn_pair_finish(prev[0], prev[1], pava, pavb)
                if prev[1] == 1:
                    flb = prev[0]
                    pending += [
                        (lambda flb=flb, lt=lt, ns=ns:
                         outproj_tile(flb, lt, ns))
                        for lt in range(4) for ns in range(4)
                    ]
                prev = (lb, p, e_list)
            # tail: A·V of the last stream (PE-dense, no pops), normalize,
            # then leftover fillers (they cover the final chain), then the
            # last out-projection
            pava, pavb = attn_pair_start()
            filler.extend(pending)
            for m in range(MC):
                av_chunk(prev[1], m, pava, pavb, prev[2][m])
            attn_pair_finish(prev[0], prev[1], pava, pavb)
            for f in filler:
                f()
            for lt in range(4):
                for ns in range(4):
                    outproj_tile(NLB - 1, lt, ns)

    nc.compile()
    return nc


def make_core_inputs(X, Wq_w, Wq_b, Wk_w, Wk_b, Wv_w, Wv_b, Wo_w):
    """Host-side sharding: per-core input dicts (shared X + per-core weights)."""
    X = np.asarray(X, np.float32)
    bf = ml_dtypes.bfloat16
    f8 = ml_dtypes.float8_e4m3fn
    xt = np.ascontiguousarray(X.T)
    xt_bf = np.ascontiguousarray(
        xt.reshape(KO, 128, L).transpose(1, 0, 2)).astype(bf).reshape(
            128, KO, NLB, LB)
    xt_f8 = np.ascontiguousarray(
        xt.reshape(KP, 2, 128, L).transpose(2, 0, 1, 3)).astype(f8)
    in_maps = []
    for c in range(NCORES):
        idx = np.array([d * NH + h for h in range(c * HPC, (c + 1) * HPC)
                        for d in range(DH)], np.int64)

        def kxj8(w):
            # (D_in, JC) -> (128, KP, 2, JC): [p,k,i,j] = 8*w.T[(2k+i)*128+p, j]
            wt = np.ascontiguousarray(
                (np.asarray(w, np.float32)[idx, :] * W8S).T)
            return np.ascontiguousarray(
                wt.reshape(KP, 2, 128, JC).transpose(2, 0, 1, 3)).astype(f8)

        def kxj(w):
            wt = np.ascontiguousarray(np.asarray(w, np.float32)[idx, :].T)
            return np.ascontiguousarray(
                wt.reshape(KO, 128, JC).transpose(1, 0, 2)).astype(bf)

        wo = np.ascontiguousarray(np.asarray(Wo_w, np.float32)[:, idx].T)
        wo = np.ascontiguousarray(
            wo.reshape(2, 128, D).transpose(1, 0, 2)).astype(bf)

        def bcol(b, s):
            return np.ascontiguousarray(
                (np.asarray(b, np.float32)[idx] * s).reshape(2, 128).T)

        in_maps.append({
            "XT8": xt_f8, "XT": xt_bf,
            "WQ8": kxj8(Wq_w), "WK8": kxj8(Wk_w), "WV": kxj(Wv_w),
            "WO": wo,
            "BQ": bcol(Wq_b, float(D) ** -0.5), "BK": bcol(Wk_b, 1.0),
            "BV": np.asarray(Wv_b, np.float32)[idx].reshape(1, JC).astype(bf),
            "ONES": np.ones((1, 128), bf),
        })
    return in_maps


_prog_cache = {}


def kernel(X, Wq_w, Wq_b, Wk_w, Wk_b, Wv_w, Wv_b, Wo_w, Wo_b, _trace=False):
    from concourse.bass_utils import run_bass_kernel_spmd

    if "nc" not in _prog_cache:
        _prog_cache["nc"] = build_program()
    nc = _prog_cache["nc"]
    in_maps = make_core_inputs(X, Wq_w, Wq_b, Wk_w, Wk_b, Wv_w, Wv_b, Wo_w)
    res = run_bass_kernel_spmd(nc, in_maps, core_ids=list(range(NCORES)),
                               trace=_trace)
    y = np.zeros((L, D), np.float64)
    for r in res.results:
        y += np.asarray(r["Y"]).astype(np.float64)
    y += np.asarray(Wo_b, np.float32).astype(np.float64)
    out = y.astype(np.float32)
    if _trace:
        kernel.last_results = res
    return out


# revision 4
# speedup vs baseline: 1.0677x; 1.0238x over previous
"""Trainium2 Bass kernel for nn_MultiHeadAttention_62319975465542 (v4).

Tensor-parallel over heads (Megatron-style): 32 heads sharded 4-per-core
across 8 NeuronCores; host sums the 8 partial out-projections + bias.

v5 over v3:
- Q/K projections run in fp8e4m3 with MatmulPerfMode.DoubleRow (2
  k-tiles per pass, 0.5 cycles/row): 4x fewer PE cycles there. Scores /
  A*V / V / out-projection stay bf16: measured on silicon, an
  idle-punctuated PE (which an fp8-fast score phase causes, since exp on
  ScalarE then paces the pipeline) drops the PE DVFS ladder from 2.4GHz
  to ~2.0GHz and taxes ALL matmuls 20%, so bf16 scores that keep the PE
  saturated are net faster.
- Score scale 1/sqrt(D) is folded into the Q store (mult+add
  tensor_scalar); weights host-scaled by 8 for fp8 normal range.
- exp is emitted earlier (scores before V-proj in the lead-in loop).
- Y partials output in bf16 (halves output DMA).
"""

import numpy as np
import ml_dtypes

import concourse.bass as bass
import concourse.tile as tile
import concourse.mybir as mybir
from concourse import bacc

F32 = mybir.dt.float32
BF16 = mybir.dt.bfloat16
FP8 = mybir.dt.float8e4
DR = mybir.MatmulPerfMode.DoubleRow
Identity = mybir.ActivationFunctionType.Identity
Exp = mybir.ActivationFunctionType.Exp
MULT = mybir.AluOpType.mult
ADD = mybir.AluOpType.add

L = 2048          # sequence length
D = 2048          # d_model
NH = 32           # total heads
DH = 64           # head dim
NCORES = 8
HPC = NH // NCORES   # heads per core = 4
JC = HPC * DH        # per-core projected width = 256
LB = 512             # l-block width
NLB = L // LB        # 4
KP = 8               # fp8 contraction k-pair chunks (2048 = 8 * 2 * 128)
KO = D // 128        # 16 bf16 contraction chunks (V proj)
MC = L // 128        # 16 key chunks
W8S = 8.0            # fp8 weights host-scaled by 8 for normal range
QS = float(D) ** -0.5 / W8S   # q store scale: 1/sqrt(D) / 8
KS = 1.0 / W8S


def build_program():
    nc = bacc.Bacc("TRN2", target_bir_lowering=False, debug=False)

    xt8_d = nc.dram_tensor("XT8", (128, KP, 2, L), FP8, kind="ExternalInput")
    xt_d = nc.dram_tensor("XT", (128, KO, NLB, LB), BF16, kind="ExternalInput")
    wq8_d = nc.dram_tensor("WQ8", (128, KP, 2, JC), FP8, kind="ExternalInput")
    wk8_d = nc.dram_tensor("WK8", (128, KP, 2, JC), FP8, kind="ExternalInput")
    wv_d = nc.dram_tensor("WV", (128, KO, JC), BF16, kind="ExternalInput")
    wo_d = nc.dram_tensor("WO", (128, 2, D), BF16, kind="ExternalInput")
    bq_d = nc.dram_tensor("BQ", (128, 2), F32, kind="ExternalInput")
    bk_d = nc.dram_tensor("BK", (128, 2), F32, kind="ExternalInput")
    bv_d = nc.dram_tensor("BV", (1, JC), BF16, kind="ExternalInput")
    ones_d = nc.dram_tensor("ONES", (1, 128), BF16, kind="ExternalInput")
    y_d = nc.dram_tensor("Y", (L, D), BF16, kind="ExternalOutput")

    with tile.TileContext(nc) as tc, nc.allow_low_precision(
            reason="fp8 QK path + bf16 V path are within tolerance"):
        with (
            tc.tile_pool(name="const", bufs=1) as cp,
            tc.tile_pool(name="xtp", bufs=2) as xtpool,
            tc.tile_pool(name="epool", bufs=20) as epool,
            tc.tile_pool(name="norm", bufs=1) as normp,
            tc.tile_pool(name="ysb", bufs=4) as ypool,
            tc.tile_pool(name="scps", bufs=2, space="PSUM") as scps,
            tc.tile_pool(name="accps", bufs=4, space="PSUM") as accps,
        ):
            wq8_sb = cp.tile((128, KP, 2, JC), FP8)
            wk8_sb = cp.tile((128, KP, 2, JC), FP8)
            wv_sb = cp.tile((128, KO, JC), BF16)
            wo_sb = cp.tile((128, 2, D), BF16)
            bq_sb = cp.tile((128, 2), F32)
            bk_sb = cp.tile((128, 2), F32)
            bv_sb = cp.tile((1, JC), BF16)
            ones_sb = cp.tile((1, 128), BF16)
            xt8_sb = cp.tile((128, KP, 2, L), FP8)
            xtb = {}    # rotating per-l-block bf16 X tiles (V-proj only)
            # qt/kt: pair p holds heads {2p, 2p+1} as partition halves
            qt_sb = [cp.tile((128, L), BF16, name=f"qt{p}") for p in range(2)]
            kt_sb = [cp.tile((128, L), BF16, name=f"kt{p}") for p in range(2)]
            v_sb = cp.tile((128, MC, HPC * 65), BF16)
            ot_sb = [cp.tile((128, L), BF16, name=f"ot{p}") for p in range(2)]

            nc.vector.memset(v_sb[:], 1.0)

            # DMA in urgency order: the Q(lb0,jc0)/K(mb0,jc0) projections and
            # first score chunks only need WQ8+BQ8+xt8(lb0)+WK8+BK8 (~2.5MB).
            def load_xt8(lb, eng):
                for k in range(0, KP, 2):
                    eng.dma_start(
                        xt8_sb[:, k:k + 2, :, lb * LB:(lb + 1) * LB],
                        xt8_d[:, k:k + 2, :, lb * LB:(lb + 1) * LB],
                    )

            def load_xt(lb, eng):
                xtb[lb] = xtpool.tile((128, KO, LB), BF16, name="xt_lb")
                for ko in range(0, KO, 4):
                    eng.dma_start(
                        xtb[lb][:, ko:ko + 4, :],
                        xt_d[:, ko:ko + 4, lb, :],
                    )

            # all loads on the sync (SP) hardware-DGE queue, in strict
            # priority order: SP issues ~565ns apart, queues round-robin,
            # so the first compute's bytes lead every queue.
            nc.sync.dma_start(wq8_sb[:, :, :, 0:128], wq8_d[:, :, :, 0:128])
            nc.sync.dma_start(bq_sb[:], bq_d[:])
            load_xt8(0, nc.sync)
            nc.sync.dma_start(wk8_sb[:, :, :, 0:128], wk8_d[:, :, :, 0:128])
            nc.sync.dma_start(bk_sb[:], bk_d[:])
            nc.sync.dma_start(wq8_sb[:, :, :, 128:256],
                              wq8_d[:, :, :, 128:256])
            nc.sync.dma_start(wk8_sb[:, :, :, 128:256],
                              wk8_d[:, :, :, 128:256])
            nc.sync.dma_start(wv_sb[:], wv_d[:])
            nc.sync.dma_start(bv_sb[:], bv_d[:])
            nc.sync.dma_start(ones_sb[:], ones_d[:])
            load_xt(0, nc.sync)
            load_xt8(1, nc.sync)
            load_xt(1, nc.sync)
            for lb in range(2, NLB):
                load_xt8(lb, nc.sync)
                load_xt(lb, nc.sync)
            nc.sync.dma_start(wo_sb[:], wo_d[:])

            def proj_qk8(w_sb, b_sb, dst, lb, jc, scale):
                """fp8 DoubleRow projection of one (jc, lb) block; PSUM ->
                bf16 store with x*scale + bias (bias pre-scaled on host)."""
                ps = accps.tile((128, LB), F32, name="acc_ps")
                for k in range(KP):
                    nc.tensor.matmul(
                        ps[:],
                        w_sb[:, k, :, jc * 128:(jc + 1) * 128],
                        xt8_sb[:, k, :, lb * LB:(lb + 1) * LB],
                        start=(k == 0), stop=(k == KP - 1),
                        perf_mode=DR,
                    )
                nc.vector.tensor_scalar(
                    dst[jc][:, lb * LB:(lb + 1) * LB], ps[:],
                    scale, b_sb[:, jc:jc + 1], MULT, ADD,
                )

            def proj_v(lb):
                """V in (l, j) layout; bias via K=1 ones-row matmul."""
                for lt in range(4):
                    vp = accps.tile((128, LB), F32, name="acc_ps")
                    for ko in range(KO):
                        nc.tensor.matmul(
                            vp[:, 0:JC],
                            xtb[lb][:, ko, lt * 128:(lt + 1) * 128],
                            wv_sb[:, ko, :],
                            start=(ko == 0), stop=False,
                        )
                    nc.tensor.matmul(
                        vp[:, 0:JC], ones_sb[0:1, :], bv_sb[0:1, :],
                        start=False, stop=True,
                    )
                    nc.vector.tensor_copy(
                        v_sb[:, lb * 4 + lt, :].rearrange(
                            "p (h e) -> p h e", h=HPC)[:, :, 0:DH],
                        vp[:, 0:JC].rearrange("p (h d) -> p h d", h=HPC))

            def attn_pair_start():
                ava = accps.tile((128, LB), F32, name="acc_ps")
                avb = accps.tile((128, LB), F32, name="acc_ps")
                return ava, avb

            def score_chunk(lb, p, m):
                """bf16 scores + exp for one (m, l-block) chunk; returns
                the bf16 exp tile (consumed by a later A·V)."""
                lsl = slice(lb * LB, (lb + 1) * LB)
                msl = slice(m * 128, (m + 1) * 128)
                sc = scps.tile((128, 2 * LB), F32, name="sc_ps")
                nc.tensor.matmul(
                    sc[:, 0:LB],
                    kt_sb[p][0:64, msl], qt_sb[p][0:64, lsl],
                )
                nc.tensor.matmul(
                    sc[:, LB:2 * LB],
                    kt_sb[p][64:128, msl], qt_sb[p][64:128, lsl],
                )
                e = epool.tile((128, 2 * LB), BF16, name="e_sb")
                nc.scalar.activation(e[:], sc[:], Exp)
                return e

            def av_chunk(p, m, ava, avb, e):
                # A·V per head, [V_h | 1] lhsT: row 64 = denominator
                nc.tensor.matmul(
                    ava[0:65, :],
                    v_sb[:, m, (2 * p) * 65:(2 * p) * 65 + 65],
                    e[:, 0:LB],
                    start=(m == 0), stop=(m == MC - 1),
                )
                nc.tensor.matmul(
                    avb[0:65, :],
                    v_sb[:, m, (2 * p + 1) * 65:(2 * p + 1) * 65 + 65],
                    e[:, LB:2 * LB],
                    start=(m == 0), stop=(m == MC - 1),
                )

            def attn_pair_finish(lb, p, ava, avb):
                # copy raw A·V (+denominator row) to SBUF, freeing the
                # PSUM accumulators immediately; normalize from SBUF
                lsl = slice(lb * LB, (lb + 1) * LB)
                sva = normp.tile((65, LB), F32, name="sva_sb")
                svb = normp.tile((65, LB), F32, name="svb_sb")
                nc.vector.tensor_copy(sva[:], ava[0:65, :])
                nc.vector.tensor_copy(svb[:], avb[0:65, :])
                # one reciprocal covers both denominators: rows staged at
                # partitions 0 (from sva) and 64 (from svb); DVE op cost is
                # free-size driven so (65,LB) costs the same as (1,LB).
                # partition_broadcast sources must sit at partition 0, so
                # rr[64] is re-staged into rb first.
                dn = normp.tile((65, LB), F32, name="dn_sb")
                nc.vector.tensor_copy(dn[0:1, :], sva[64:65, :])
                nc.vector.tensor_copy(dn[64:65, :], svb[64:65, :])
                rr = normp.tile((65, LB), F32, name="rr_sb")
                nc.vector.reciprocal(rr[:], dn[:])
                rb = normp.tile((1, LB), F32, name="rb_sb")
                nc.vector.tensor_copy(rb[:], rr[64:65, :])
                rba = normp.tile((64, LB), F32, name="rba_sb")
                rbb = normp.tile((64, LB), F32, name="rbb_sb")
                nc.gpsimd.partition_broadcast(rba[:], rr[0:1, :])
                nc.gpsimd.partition_broadcast(rbb[:], rb[:])
                nc.vector.tensor_tensor(
                    ot_sb[p][0:64, lsl], sva[0:64, :], rba[:], MULT)
                nc.vector.tensor_tensor(
                    ot_sb[p][64:128, lsl], svb[0:64, :], rbb[:], MULT)

            def outproj_tile(lb, lt, ns, pool=None, eng=None):
                row0 = lb * LB + lt * 128
                yp = (pool or accps).tile((128, 512), F32, name="acc_ps")
                for jc in range(2):
                    nc.tensor.matmul(
                        yp[:],
                        ot_sb[jc][:, row0:row0 + 128],
                        wo_sb[:, jc, ns * 512:(ns + 1) * 512],
                        start=(jc == 0), stop=(jc == 1),
                    )
                ty = ypool.tile((128, 512), BF16, name="y_sb")
                if eng is nc.scalar:
                    nc.scalar.copy(ty[:], yp[:])
                else:
                    nc.vector.tensor_copy(ty[:], yp[:])
                nc.sync.dma_start(
                    y_d[row0:row0 + 128, ns * 512:(ns + 1) * 512], ty[:])

            # ── Emission schedule ────────────────────────────────────────
            # Streams S_i = (lb, p). Scores of stream i overlap A·V of
            # stream i-1 on PE; exp (ScalarE) is the pacing engine, so
            # scores are emitted as early as the projections allow.
            ava0, avb0 = attn_pair_start()
            e1 = []
            proj_qk8(wq8_sb, bq_sb, qt_sb, 0, 0, QS)
            for mb in range(NLB):
                proj_qk8(wk8_sb, bk_sb, kt_sb, mb, 0, KS)
                s0 = [score_chunk(0, 0, m) for m in range(4 * mb, 4 * mb + 4)]
                if mb == 0:
                    proj_qk8(wq8_sb, bq_sb, qt_sb, 0, 1, QS)
                proj_qk8(wk8_sb, bk_sb, kt_sb, mb, 1, KS)
                e1 += [score_chunk(0, 1, m) for m in range(4 * mb, 4 * mb + 4)]
                proj_v(mb)
                for i, m in enumerate(range(4 * mb, 4 * mb + 4)):
                    av_chunk(0, m, ava0, avb0, s0[i])
            attn_pair_finish(0, 0, ava0, avb0)

            # Q(lb1) must be resident before stream (1,0)'s scores
            proj_qk8(wq8_sb, bq_sb, qt_sb, 1, 0, QS)
            proj_qk8(wq8_sb, bq_sb, qt_sb, 1, 1, QS)

            filler = [
                (lambda lb=lb, jc=jc: proj_qk8(wq8_sb, bq_sb, qt_sb, lb, jc, QS))
                for lb in range(2, NLB) for jc in range(2)
            ]
            prev = (0, 1, e1)    # stream whose A·V phase is pending
            pending = []         # fillers gated on a not-yet-finished chain
            streams = [(lb, p) for lb in range(1, NLB) for p in range(2)]
            for i, (lb, p) in enumerate(streams):
                last = i == len(streams) - 1
                e_list = []
                pava, pavb = attn_pair_start()
                for m in range(MC):
                    if m == 12 and pending:
                        # one half-iteration after the finish that queued
                        # them: the normalize chain has drained by now, so
                        # these no longer stall the in-order PE queue
                        filler.extend(pending)
                        pending = []
                    e_list.append(score_chunk(lb, p, m))
                    av_chunk(prev[1], m, pava, pavb, prev[2][m])
                    if filler and (last or m % 2 == 0):
                        filler.pop(0)()
                attn_pair_finish(prev[0], prev[1], pava, pavb)
                if prev[1] == 1:
                    flb = prev[0]
                    pending += [
                        (lambda flb=flb, lt=lt, ns=ns:
                         outproj_tile(flb, lt, ns))
                        for lt in range(4) for ns in range(4)
                    ]
                prev = (lb, p, e_list)
            # tail: A·V of the last stream (PE-dense, no pops), normalize,
            # then leftover fillers (they cover the final chain), then the
            # last out-projection
            pava, pavb = attn_pair_start()
            filler.extend(pending)
            for m in range(MC):
                av_chunk(prev[1], m, pava, pavb, prev[2][m])
            attn_pair_finish(prev[0], prev[1], pava, pavb)
            tengs = [nc.vector, nc.scalar]
            ti = 0
            for f in filler:
                f()
            for lt in range(4):
                for ns in range(4):
                    outproj_tile(NLB - 1, lt, ns, eng=tengs[ti % 2])
                    ti += 1

    nc.compile()
    return nc


def make_core_inputs(X, Wq_w, Wq_b, Wk_w, Wk_b, Wv_w, Wv_b, Wo_w):
    """Host-side sharding: per-core input dicts (shared X + per-core weights)."""
    X = np.asarray(X, np.float32)
    bf = ml_dtypes.bfloat16
    f8 = ml_dtypes.float8_e4m3fn
    xt = np.ascontiguousarray(X.T)
    xt_bf = np.ascontiguousarray(
        xt.reshape(KO, 128, L).transpose(1, 0, 2)).astype(bf).reshape(
            128, KO, NLB, LB)
    xt_f8 = np.ascontiguousarray(
        xt.reshape(KP, 2, 128, L).transpose(2, 0, 1, 3)).astype(f8)
    in_maps = []
    for c in range(NCORES):
        idx = np.array([d * NH + h for h in range(c * HPC, (c + 1) * HPC)
                        for d in range(DH)], np.int64)

        def kxj8(w):
            # (D_in, JC) -> (128, KP, 2, JC): [p,k,i,j] = 8*w.T[(2k+i)*128+p, j]
            wt = np.ascontiguousarray(
                (np.asarray(w, np.float32)[idx, :] * W8S).T)
            return np.ascontiguousarray(
                wt.reshape(KP, 2, 128, JC).transpose(2, 0, 1, 3)).astype(f8)

        def kxj(w):
            wt = np.ascontiguousarray(np.asarray(w, np.float32)[idx, :].T)
            return np.ascontiguousarray(
                wt.reshape(KO, 128, JC).transpose(1, 0, 2)).astype(bf)

        wo = np.ascontiguousarray(np.asarray(Wo_w, np.float32)[:, idx].T)
        wo = np.ascontiguousarray(
            wo.reshape(2, 128, D).transpose(1, 0, 2)).astype(bf)

        def bcol(b, s):
            return np.ascontiguousarray(
                (np.asarray(b, np.float32)[idx] * s).reshape(2, 128).T)

        in_maps.append({
            "XT8": xt_f8, "XT": xt_bf,
            "WQ8": kxj8(Wq_w), "WK8": kxj8(Wk_w), "WV": kxj(Wv_w),
            "WO": wo,
            "BQ": bcol(Wq_b, float(D) ** -0.5), "BK": bcol(Wk_b, 1.0),
            "BV": np.asarray(Wv_b, np.float32)[idx].reshape(1, JC).astype(bf),
            "ONES": np.ones((1, 128), bf),
        })
    return in_maps


_prog_cache = {}


def kernel(X, Wq_w, Wq_b, Wk_w, Wk_b, Wv_w, Wv_b, Wo_w, Wo_b, _trace=False):
    from concourse.bass_utils import run_bass_kernel_spmd

    if "nc" not in _prog_cache:
        _prog_cache["nc"] = build_program()
    nc = _prog_cache["nc"]
    in_maps = make_core_inputs(X, Wq_w, Wq_b, Wk_w, Wk_b, Wv_w, Wv_b, Wo_w)
    res = run_bass_kernel_spmd(nc, in_maps, core_ids=list(range(NCORES)),
                               trace=_trace)
    y = np.zeros((L, D), np.float64)
    for r in res.results:
        y += np.asarray(r["Y"]).astype(np.float64)
    y += np.asarray(Wo_b, np.float32).astype(np.float64)
    out = y.astype(np.float32)
    if _trace:
        kernel.last_results = res
    return out
